# revision 1
# baseline (speedup 1.0000x reference)
"""Local window attention (7x7 windows, 8 heads, d=64) Trainium2 Bass kernel.

Full inputs in, full outputs out. Internally: data-parallel over batch across
8 NeuronCores (4 images per core). All shapes hardcoded per the problem spec:
  fmap (32, 56, 56, 256) f32, Wq (256,512), Wkv (256,1024), Wo (512,256), bo (256,)

V2: all matmuls in bf16 (inputs cast on host, fp32 PSUM accumulate), output
written bf16 and upcast on host; one merged DMA per 2-window group each way.

Per-core dataflow (one "group" = 2 adjacent-y windows = 98 tokens, padded to
2x64 token slots on partitions so window w sits at partitions 64w..64w+48):
  f_raw [128,256]  <- DMA (2 windows, one dma_start)
  fT    [128,2,128](PE transpose)  c-on-partition
  qT,kT [128,4,98] = W.T @ fT      (4 n-chunks of 128, tokens compact 2x49)
  v     [128,512]  = f @ Wv        (token-padded rows)
  ST    [128,4,2,49] psum: per (chunk,hp,w): kT.T @ qT -> S^T [j,i]
  expS  = exp(SCALE * ST)          one ACT op per window
  out'  [128,2,2,65] psum x2: expS.T @ [v | ones] -> [i, 64+denom]
  out   [128,512] = out' * recip(denom)  (token-padded rows, head-major cols)
  outT  (PE transpose x4) -> final = outT.T @ Wo + bo -> DMA out (one dma_start)
"""

from contextlib import ExitStack

import ml_dtypes
import numpy as np

import concourse.bacc as bacc
import concourse.bass as bass
import concourse.tile as tile
from concourse import mybir
from concourse.masks import make_identity
from concourse.bass_utils import run_bass_kernel_spmd

P = 7
PP = 49          # tokens per window
H = 8            # heads
D = 64           # head dim
DIM = 256        # channels
INNER = 512      # h*d
SCALE = D ** -0.5
IMGS_PER_CORE = 4
NCORES = 8
X = 56
NW = X // P      # 8 windows per axis
FP32 = mybir.dt.float32
BF16 = mybir.dt.bfloat16
NPBF16 = ml_dtypes.bfloat16


def build_bass(n_imgs=IMGS_PER_CORE):
    nc = bacc.Bacc("TRN2", target_bir_lowering=False, debug=False)

    fm = nc.dram_tensor("fmap", [n_imgs, X, X, DIM], BF16, kind="ExternalInput").ap()
    wq = nc.dram_tensor("Wq", [DIM, INNER], BF16, kind="ExternalInput").ap()
    wkv = nc.dram_tensor("Wkv", [DIM, 2 * INNER], BF16, kind="ExternalInput").ap()
    wo = nc.dram_tensor("Wo", [INNER, DIM], BF16, kind="ExternalInput").ap()
    bo = nc.dram_tensor("bo", [DIM], BF16, kind="ExternalInput").ap()
    out = nc.dram_tensor("out", [n_imgs, X, X, DIM], BF16, kind="ExternalOutput").ap()

    with tile.TileContext(nc) as tc:
        with ExitStack() as ctx:
            build_kernel(ctx, tc, out, fm, wq, wkv, wo, bo, n_imgs)
    nc.compile()
    return nc


def build_kernel(ctx, tc, out, fm, wq, wkv, wo, bo, n_imgs=IMGS_PER_CORE):
    nc = tc.nc
    consts = ctx.enter_context(tc.tile_pool(name="consts", bufs=1))
    sb = ctx.enter_context(tc.tile_pool(name="sb", bufs=3))
    ps = ctx.enter_context(tc.tile_pool(name="ps", bufs=8, space="PSUM"))

    # ---- constants ----
    ident = consts.tile([128, 128], BF16)
    make_identity(nc, ident[:])

    ones = consts.tile([128, 128], BF16)
    nc.gpsimd.memset(ones[:], 1.0)

    # weights, contraction dim (input channels) on partitions, chunked by 128
    wq_s = consts.tile([128, 2, INNER], BF16)   # [ck, kc, n]
    nc.sync.dma_start(out=wq_s[:], in_=wq.rearrange("(kc ck) n -> ck kc n", ck=128))
    wk_s = consts.tile([128, 2, INNER], BF16)
    nc.sync.dma_start(out=wk_s[:], in_=wkv[:, 0:INNER].rearrange("(kc ck) n -> ck kc n", ck=128))
    wv_s = consts.tile([128, 2, INNER], BF16)
    nc.sync.dma_start(out=wv_s[:], in_=wkv[:, INNER:2 * INNER].rearrange("(kc ck) n -> ck kc n", ck=128))
    wo_s = consts.tile([128, 4, DIM], BF16)     # [ck, kc, m]
    nc.sync.dma_start(out=wo_s[:], in_=wo.rearrange("(kc ck) m -> ck kc m", ck=128))
    bo_s = consts.tile([1, DIM], BF16)
    nc.sync.dma_start(out=bo_s[:], in_=bo[None, :])

    # ---- main loop: one group = 2 windows (same wx, adjacent wy) ----
    for img in range(n_imgs):
        for wx in range(NW):
            for u in range(NW // 2):
                group(nc, sb, ps, out, fm, wq_s, wk_s, wv_s, wo_s, bo_s, ident, ones,
                      img, wx, u)


def group(nc, sb, ps, out, fm, wq_s, wk_s, wv_s, wo_s, bo_s, ident, ones, img, wx, u):
    # 1. load 2 windows, token-padded: window w tokens at partitions 64w..64w+48
    #    one dma_start: partition dims (w, r, dy) with w-stride 64, free = c
    f_raw = sb.tile([128, DIM], BF16, tag="f_raw")
    for w in range(2):
        wy = 2 * u + w
        nc.sync.dma_start(
            out=f_raw[64 * w:64 * w + PP, :],
            in_=fm[img, P * wx:P * wx + P, P * wy:P * wy + P, :])

    def ps_tile(shape, dtype=FP32):
        # uniform bank-sized psum slots; view-slice to the requested shape
        t = ps.tile([128, 512 * 4 // mybir.dt.size(dtype)], dtype, tag="ps")
        n = int(np.prod(shape[1:]))
        v_ = t[:, 0:n]
        if len(shape) > 2:
            dims = " ".join(f"d{i}" for i in range(1, len(shape)))
            v_ = v_.rearrange(f"p ({dims}) -> p {dims}",
                              **{f"d{i}": shape[i] for i in range(1, len(shape) - 1)})
        return v_

    # 2-3. transpose -> fT [ck, kc, t]  (c on partitions, tokens padded on free)
    fT_ps = ps_tile([128, 2, 128], BF16)
    for kc in range(2):
        nc.tensor.transpose(fT_ps[:, kc, :], f_raw[:, 128 * kc:128 * kc + 128], ident[:])
    fT = sb.tile([128, 2, 128], BF16, tag="fT")
    nc.scalar.copy(fT[:], fT_ps[:])

    # 4-5. qT, kT [nc*128, 2x64 padded] = W.T @ fT
    qT_ps = ps_tile([128, 4, 128])
    kT_ps = ps_tile([128, 4, 128])
    for nk in range(4):
        for kc in range(2):
            nc.tensor.matmul(qT_ps[:, nk, :], wq_s[:, kc, 128 * nk:128 * nk + 128],
                             fT[:, kc, :], start=(kc == 0), stop=(kc == 1))
            nc.tensor.matmul(kT_ps[:, nk, :], wk_s[:, kc, 128 * nk:128 * nk + 128],
                             fT[:, kc, :], start=(kc == 0), stop=(kc == 1))
    # HW bug: matmul operands must start at partition 0 (high-half streaming
    # is broken), so split head-parities into base-0 tiles during the
    # mandatory psum->SBUF copies.
    qT = sb.tile([64, 4, 2, 128], BF16, tag="qT")   # [d, ch, hp, t]
    kT = sb.tile([64, 4, 2, 128], BF16, tag="kT")
    for hp in range(2):
        nc.vector.tensor_copy(qT[:, :, hp, :], qT_ps[64 * hp:64 * hp + 64, :, :])
        nc.scalar.copy(kT[:, :, hp, :], kT_ps[64 * hp:64 * hp + 64, :, :])

    # 6-7. v [t(padded), 512] = f @ Wv
    v_ps = ps_tile([128, INNER])
    for kc in range(2):
        nc.tensor.matmul(v_ps[:], fT[:, kc, :], wv_s[:, kc, :],
                         start=(kc == 0), stop=(kc == 1))
    v = sb.tile([64, 2, INNER], BF16, tag="v")      # [j, w, n]
    for w in range(2):
        nc.vector.tensor_copy(v[:, w, :], v_ps[64 * w:64 * w + 64, :])

    # 8-9. S^T then exp:  ST[j@64w, (ch, hp, i)]  (i padded to 64)
    # lhsT = kT slice with M=64 (incl. 15 pad cols) so psum rows are fully
    # written; pad lanes carry junk that is never consumed.
    st_ps = ps_tile([128, 4, 2, 64])
    for ch in range(4):
        for hp in range(2):
            for w in range(2):
                nc.tensor.matmul(
                    st_ps[64 * w:64 * w + 64, ch, hp, :],
                    kT[:, ch, hp, 64 * w:64 * w + 64],
                    qT[:, ch, hp, 64 * w:64 * w + 64],
                    tile_position=(0, 64 * w),
                )
    expS = sb.tile([64, 2, 4, 2, 64], BF16, tag="expS")  # [j, w, ch, hp, i]
    for w in range(2):
        nc.scalar.activation(expS[:, w, :, :, :], st_ps[64 * w:64 * w + 64, :, :, :],
                             mybir.ActivationFunctionType.Exp, scale=SCALE)

    # 10. out' = expS.T @ [v | 1]:   av[i@64w, (chL, hp, d|denom)]
    av_tiles = []
    for chpair in range(2):
        av = ps_tile([128, 2, 2, D + 1])
        av_tiles.append(av)
        for chL in range(2):
            ch = 2 * chpair + chL
            for hp in range(2):
                h = 2 * ch + hp
                for w in range(2):
                    # lhsT: K = 49 real keys (base 0), M = 64 (incl. pad
                    # queries so psum rows are fully written)
                    e = expS[0:PP, w, ch, hp, :]
                    nc.tensor.matmul(av[64 * w:64 * w + 64, chL, hp, 0:D],
                                     e, v[0:PP, w, D * h:D * h + D],
                                     tile_position=(0, 64 * w))
                    nc.tensor.matmul(av[64 * w:64 * w + 64, chL, hp, D:D + 1],
                                     e, ones[0:PP, 0:1],
                                     tile_position=(0, 64 * w))

    # 11-12. normalize: out_tok [t(padded), h*64+d]
    out_tok = sb.tile([128, INNER], BF16, tag="out_tok")
    for chpair in range(2):
        av = av_tiles[chpair]
        recd = sb.tile([128, 2, 2], FP32, tag="recd")
        nc.vector.reciprocal(recd[:], av[:, :, :, D])
        for chL in range(2):
            for hp in range(2):
                h = 2 * (2 * chpair + chL) + hp
                nc.vector.tensor_scalar(
                    out=out_tok[:, D * h:D * h + D],
                    in0=av[:, chL, hp, 0:D],
                    scalar1=recd[:, chL, hp:hp + 1],
                    scalar2=None,
                    op0=mybir.AluOpType.mult,
                )

    # 13-14. transpose out_tok -> outT [n, t(padded)]
    ot_ps = ps_tile([128, 4, 128], BF16)
    for nk in range(4):
        nc.tensor.transpose(ot_ps[:, nk, :], out_tok[:, 128 * nk:128 * nk + 128],
                            ident[:])
    outT = sb.tile([128, 4, 128], BF16, tag="outT")
    nc.scalar.copy(outT[:], ot_ps[:])

    # 15. final = outT.T @ Wo + bo   [t(padded), 256]
    fin_ps = ps_tile([128, DIM])
    for nk in range(4):
        nc.tensor.matmul(fin_ps[:], outT[:, nk, :], wo_s[:, nk, :],
                         start=(nk == 0), stop=False)
    nc.tensor.matmul(fin_ps[:], ones[0:1, 0:128], bo_s[:], start=False, stop=True)
    fin = sb.tile([128, DIM], BF16, tag="fin")
    nc.vector.tensor_copy(fin[:], fin_ps[:])

    # 16. store: one dma_start, mirror of the load pattern
    for w in range(2):
        wy = 2 * u + w
        nc.sync.dma_start(
            out=out[img, P * wx:P * wx + P, P * wy:P * wy + P, :],
            in_=fin[64 * w:64 * w + PP, :])


_CACHED = {}


def _get_nc():
    if "nc" not in _CACHED:
        _CACHED["nc"] = build_bass()
    return _CACHED["nc"]


def kernel(fmap, Wq, Wkv, Wo, bo, _trace=False, _trace_kwargs=None):
    fmap = np.ascontiguousarray(fmap).astype(NPBF16)
    Wq = np.ascontiguousarray(Wq).astype(NPBF16)
    Wkv = np.ascontiguousarray(Wkv).astype(NPBF16)
    Wo = np.ascontiguousarray(Wo).astype(NPBF16)
    bo = np.ascontiguousarray(bo).astype(NPBF16)
    nc = _get_nc()
    in_maps = []
    for c in range(NCORES):
        in_maps.append({
            "fmap": fmap[IMGS_PER_CORE * c:IMGS_PER_CORE * (c + 1)],
            "Wq": Wq, "Wkv": Wkv, "Wo": Wo, "bo": bo,
        })
    res = run_bass_kernel_spmd(nc, in_maps, core_ids=list(range(NCORES)),
                               trace=_trace, **(_trace_kwargs or {}))
    outs = [r["out"].astype(np.float32) for r in res.results]
    full = np.concatenate(outs, axis=0)
    if _trace:
        return full, res
    return full



# revision 4
# speedup vs baseline: 1.5754x; 1.5754x over previous
"""Local window attention (7x7 windows, 8 heads, d=64) Trainium2 Bass kernel, v3.

Full inputs in, full outputs out. Data-parallel over batch: 4 images/core x 8.
  fmap (32,56,56,256) f32, Wq (256,512), Wkv (256,1024), Wo (512,256), bo (256,)

v3 dataflow (channel-major, host pre-transposed):
  Host packs fmap to fmp[img, ck, c128, wx, wy, t64] bf16 (t = p1*7+p2, 64-slot
  padded windows, zeros in pad) so the kernel DMAs f^T directly - no on-chip
  input transposes.
  Per (img, wx) tile (512 padded tokens = 8 windows = 4 window-pairs):
    qT/kT [n128 x4, t512] = Wq/Wk chunk.T @ fT      (16 MMs, N=512)
    v     [t128, n512] per pair = fT-slice.T @ Wv   (8 MMs, N=512)
    per pair: S^T pair-batched [j128, hp, nk, i128] (8 MMs, N=128,
        row-tiled (64*hp,0); off-diagonal window blocks are garbage, unused)
      exp of the two diagonal blocks (2 ACTs) -> expS [j128, hp, nk, i64] bf16
      AV: av[i64w.., h, 0:65] = expS_w.T @ [v|1]    (16 MMs, N=65,
        quadrant-tiled (64w,64w); col 64 = softmax denominator)
      out_tok = av * recip(denom) broadcast        (1 recip + 1 tensor_tensor)
      outT = PE-transpose(out_tok)                 (4 transposes)
      fin [t128, 256] = outT.T @ Wo + bo           (5 MMs, N=256) -> DMA out
  Host unpacks [img, wx, u, t128, c] bf16 -> (32,56,56,256) f32.
"""

from contextlib import ExitStack

import ml_dtypes
import numpy as np

import concourse.bacc as bacc
import concourse.bass as bass
import concourse.tile as tile
from concourse import mybir
from concourse.masks import make_identity
from concourse.bass_utils import run_bass_kernel_spmd

P = 7
PP = 49
H = 8
D = 64
DIM = 256
INNER = 512
SCALE = D ** -0.5
IMGS_PER_CORE = 4
NCORES = 8
X = 56
NW = X // P      # 8 windows per axis
FP32 = mybir.dt.float32
BF16 = mybir.dt.bfloat16
NPBF16 = ml_dtypes.bfloat16
Exp = mybir.ActivationFunctionType.Exp


def build_bass(n_imgs=IMGS_PER_CORE):
    nc = bacc.Bacc("TRN2", target_bir_lowering=False, debug=False)

    fmp = nc.dram_tensor("fmp", [n_imgs, 2, 128, NW, NW, 64], BF16,
                         kind="ExternalInput").ap()
    wq = nc.dram_tensor("Wq", [2, 128, INNER], BF16, kind="ExternalInput").ap()
    wk = nc.dram_tensor("Wk", [2, 128, INNER], BF16, kind="ExternalInput").ap()
    wv = nc.dram_tensor("Wv", [2, 128, INNER], BF16, kind="ExternalInput").ap()
    wo = nc.dram_tensor("Wo", [4, 128, DIM], BF16, kind="ExternalInput").ap()
    bo = nc.dram_tensor("bo", [DIM], BF16, kind="ExternalInput").ap()
    out = nc.dram_tensor("out", [n_imgs, NW, 4, 128, DIM], BF16,
                         kind="ExternalOutput").ap()

    with tile.TileContext(nc) as tc:
        with ExitStack() as ctx:
            build_kernel(ctx, tc, out, fmp, wq, wk, wv, wo, bo, n_imgs)
    nc.compile()
    return nc


def build_kernel(ctx, tc, out, fmp, wq, wk, wv, wo, bo, n_imgs):
    nc = tc.nc
    consts = ctx.enter_context(tc.tile_pool(name="consts", bufs=1))
    sb = ctx.enter_context(tc.tile_pool(name="sb", bufs=3))
    ps = ctx.enter_context(tc.tile_pool(name="ps", bufs=4, space="PSUM"))

    ident = consts.tile([128, 128], BF16)
    make_identity(nc, ident[:])
    ones = consts.tile([1, 128], BF16)
    nc.gpsimd.memset(ones[:], 1.0)

    wq_s = consts.tile([128, 2, INNER], BF16)
    nc.sync.dma_start(out=wq_s[:], in_=wq.rearrange("kc ck n -> ck kc n"))
    wk_s = consts.tile([128, 2, INNER], BF16)
    nc.sync.dma_start(out=wk_s[:], in_=wk.rearrange("kc ck n -> ck kc n"))
    wv_s = consts.tile([128, 2, INNER], BF16)
    nc.sync.dma_start(out=wv_s[:], in_=wv.rearrange("kc ck n -> ck kc n"))
    wo_s = consts.tile([128, 4, DIM], BF16)
    nc.sync.dma_start(out=wo_s[:], in_=wo.rearrange("kc ck m -> ck kc m"))
    bo_s = consts.tile([1, DIM], BF16)
    nc.sync.dma_start(out=bo_s[:], in_=bo[None, :])

    def ps_slot(shape, dtype=FP32):
        # uniform 2-bank (4KB/partition) psum slots; view-slice to shape
        t = ps.tile([128, 4096 // mybir.dt.size(dtype)], dtype, tag="ps")
        n = int(np.prod(shape[1:]))
        v_ = t[:, 0:n]
        if len(shape) > 2:
            dims = " ".join(f"d{i}" for i in range(1, len(shape)))
            v_ = v_.rearrange(f"p ({dims}) -> p {dims}",
                              **{f"d{i}": shape[i] for i in range(1, len(shape) - 1)})
        return v_

    for img in range(n_imgs):
        for wx in range(NW):
            tile_iter(nc, sb, ps_slot, out, fmp, wq_s, wk_s, wv_s, wo_s, bo_s,
                      ident, ones, img, wx)


def tile_iter(nc, sb, ps_slot, out, fmp, wq_s, wk_s, wv_s, wo_s, bo_s,
              ident, ones, img, wx):
    # ---- load fT [c128, ck, t512] ----
    fT = sb.tile([128, 2, 512], BF16, tag="fT")
    nc.sync.dma_start(
        out=fT[:],
        in_=fmp[img, :, :, wx, :, :].rearrange("ck c wy t -> c ck (wy t)"))

    # ---- qT, kT: [n128, t512] x4 chunks ----
    qp = [ps_slot([128, 2, 512]) for _ in range(2)]   # slot holds nk, nk+1
    kp = [ps_slot([128, 2, 512]) for _ in range(2)]
    for half in range(2):
        for sub in range(2):
            nk = 2 * half + sub
            for kc in range(2):
                nc.tensor.matmul(qp[half][:, sub, :],
                                 wq_s[:, kc, 128 * nk:128 * nk + 128],
                                 fT[:, kc, :], start=(kc == 0), stop=(kc == 1))
        for sub in range(2):
            nk = 2 * half + sub
            for kc in range(2):
                nc.tensor.matmul(kp[half][:, sub, :],
                                 wk_s[:, kc, 128 * nk:128 * nk + 128],
                                 fT[:, kc, :], start=(kc == 0), stop=(kc == 1))
    qT = sb.tile([128, 4, 512], BF16, tag="qT")
    kT = sb.tile([128, 4, 512], BF16, tag="kT")
    # spread evacuation across engines
    nc.vector.tensor_copy(qT[:, 0:2, :], qp[0][:])
    nc.vector.tensor_copy(qT[:, 2:4, :], qp[1][:])
    nc.scalar.copy(kT[:, 0:2, :], kp[0][:])
    nc.scalar.copy(kT[:, 2:4, :], kp[1][:])

    # ---- v: per pair [t128, n512]; vhat adds ones col per head ----
    vhat = []
    for uh in range(2):
        vp = ps_slot([128, 2, 512])   # two pairs per slot? no: [t128, pairsub, n512]
        for us in range(2):
            u = 2 * uh + us
            for kc in range(2):
                nc.tensor.matmul(vp[:, us, :],
                                 fT[:, kc, 128 * u:128 * u + 128],
                                 wv_s[:, kc, :], start=(kc == 0), stop=(kc == 1))
        for us in range(2):
            u = 2 * uh + us
            vh = sb.tile([128, H, D + 1], BF16, tag=f"vhat{u}")
            nc.vector.tensor_copy(
                vh[:, :, 0:D], vp[:, us, :].rearrange("p (h d) -> p h d", h=H))
            nc.gpsimd.memset(vh[:, :, D:D + 1], 1.0)
            vhat.append(vh)

    # ---- per window-pair attention + output ----
    for u in range(4):
        pair_iter(nc, sb, ps_slot, out, qT, kT, vhat[u], wo_s, bo_s, ident,
                  ones, img, wx, u)


def pair_iter(nc, sb, ps_slot, out, qT, kT, vh, wo_s, bo_s, ident, ones,
              img, wx, u):
    # S^T pair-batched: sp[j128, hp, nk, i128]
    sp = ps_slot([128, 2, 4, 128])
    for nk in range(4):
        for hp in range(2):
            nc.tensor.matmul(
                sp[:, hp, nk, :],
                kT[64 * hp:64 * hp + 64, nk, 128 * u:128 * u + 128],
                qT[64 * hp:64 * hp + 64, nk, 128 * u:128 * u + 128],
                start=True, stop=True, tile_position=(64 * hp, 0))

    # exp of diagonal window blocks -> expS[j128, hp, nk, i64] bf16
    expS = sb.tile([128, 2, 4, 64], BF16, tag="expS")
    nc.scalar.activation(expS[0:64, :, :, :], sp[0:64, :, :, 0:64], Exp,
                         scale=SCALE)
    nc.scalar.activation(expS[64:128, :, :, :], sp[64:128, :, :, 64:128], Exp,
                         scale=SCALE)

    # AV + denominator: av[i, h, 0:65]
    av = ps_slot([128, H, 128])
    for h in range(H):
        nk, hp = h // 2, h % 2
        for w in range(2):
            nc.tensor.matmul(
                av[64 * w:64 * w + 64, h, 0:D + 1],
                expS[64 * w:64 * w + PP, hp, nk, :],
                vh[64 * w:64 * w + PP, h, :],
                start=True, stop=True, tile_position=(64 * w, 64 * w))

    # normalize: out_tok[i, h, d] = av * recip(denom)
    recd = sb.tile([128, H], FP32, tag="recd")
    nc.vector.reciprocal(recd[:], av[:, :, D])
    out_tok = sb.tile([128, H, D], BF16, tag="out_tok")
    nc.vector.tensor_tensor(
        out=out_tok[:], in0=av[:, :, 0:D],
        in1=recd[:].unsqueeze(2).broadcast_to([128, H, D]),
        op=mybir.AluOpType.mult)

    # transpose -> outT [n128, nk, t128]
    tp = ps_slot([128, 4, 128], BF16)
    ot2 = out_tok[:].rearrange("p h d -> p (h d)")
    for nk in range(4):
        nc.tensor.transpose(tp[:, nk, :], ot2[:, 128 * nk:128 * nk + 128],
                            ident[:])
    outT = sb.tile([128, 4, 128], BF16, tag="outT")
    nc.vector.tensor_copy(outT[:], tp[:])

    # out projection + bias
    fin = ps_slot([128, DIM])
    for nk in range(4):
        nc.tensor.matmul(fin[:], outT[:, nk, :], wo_s[:, nk, :],
                         start=(nk == 0), stop=False)
    nc.tensor.matmul(fin[:], ones[0:1, 0:128], bo_s[:], start=False, stop=True)
    fo = sb.tile([128, DIM], BF16, tag="fo")
    nc.scalar.copy(fo[:], fin[:])
    nc.sync.dma_start(out=out[img, wx, u], in_=fo[:])


_CACHED = {}


def _get_nc():
    if "nc" not in _CACHED:
        _CACHED["nc"] = build_bass()
    return _CACHED["nc"]


def _marshal_fmap(fmap):
    b = fmap.shape[0]
    A = fmap.astype(NPBF16)
    A = A.reshape(b, NW, P, NW, P, DIM).transpose(0, 5, 1, 3, 2, 4)
    A = np.ascontiguousarray(A).reshape(b, DIM, NW, NW, PP)
    T = np.zeros((b, DIM, NW, NW, 64), dtype=NPBF16)
    T[..., :PP] = A
    return T.reshape(b, 2, 128, NW, NW, 64)


def _unmarshal_out(O, b):
    # O: [b, wx, u, t2(=w*64+t), c] bf16
    O = O.reshape(b, NW, 4, 2, 64, DIM)[:, :, :, :, :PP, :]
    O = O.reshape(b, NW, NW, P, P, DIM)         # img, wx, wy(=2u+w), p1, p2, c
    O = O.transpose(0, 1, 3, 2, 4, 5).reshape(b, X, X, DIM)
    return O.astype(np.float32)


def kernel(fmap, Wq, Wkv, Wo, bo, _trace=False, _trace_kwargs=None):
    fmp = _marshal_fmap(np.ascontiguousarray(fmap))
    Wq_ = np.ascontiguousarray(Wq).astype(NPBF16).reshape(2, 128, INNER)
    Wk_ = np.ascontiguousarray(Wkv[:, :INNER]).astype(NPBF16).reshape(2, 128, INNER)
    Wv_ = np.ascontiguousarray(Wkv[:, INNER:]).astype(NPBF16).reshape(2, 128, INNER)
    Wo_ = np.ascontiguousarray(Wo).astype(NPBF16).reshape(4, 128, DIM)
    bo_ = np.ascontiguousarray(bo).astype(NPBF16)
    nc = _get_nc()
    in_maps = []
    for c in range(NCORES):
        in_maps.append({
            "fmp": fmp[IMGS_PER_CORE * c:IMGS_PER_CORE * (c + 1)],
            "Wq": Wq_, "Wk": Wk_, "Wv": Wv_, "Wo": Wo_, "bo": bo_,
        })
    res = run_bass_kernel_spmd(nc, in_maps, core_ids=list(range(NCORES)),
                               trace=_trace, **(_trace_kwargs or {}))
    outs = [_unmarshal_out(r["out"], IMGS_PER_CORE) for r in res.results]
    full = np.concatenate(outs, axis=0)
    if _trace:
        return full, res
    return full


# revision 8
# speedup vs baseline: 2.5091x; 1.5927x over previous
"""Local window attention (7x7 windows, 8 heads, d=64) Trainium2 Bass kernel, v3.

Full inputs in, full outputs out. Data-parallel over batch: 4 images/core x 8.
  fmap (32,56,56,256) f32, Wq (256,512), Wkv (256,1024), Wo (512,256), bo (256,)

v3 dataflow (channel-major, host pre-transposed):
  Host packs fmap to fmp[img, ck, c128, wx, wy, t64] bf16 (t = p1*7+p2, 64-slot
  padded windows, zeros in pad) so the kernel DMAs f^T directly - no on-chip
  input transposes.
  Per (img, wx) tile (512 padded tokens = 8 windows = 4 window-pairs):
    qT/kT [n128 x4, t512] = Wq/Wk chunk.T @ fT      (16 MMs, N=512)
    v     [t128, n512] per pair = fT-slice.T @ Wv   (8 MMs, N=512)
    per pair: S^T pair-batched [j128, hp, nk, i128] (8 MMs, N=128,
        row-tiled (64*hp,0); off-diagonal window blocks are garbage, unused)
      exp of the two diagonal blocks (2 ACTs) -> expS [j128, hp, nk, i64] bf16
      AV: av[i64w.., h, 0:65] = expS_w.T @ [v|1]    (16 MMs, N=65,
        quadrant-tiled (64w,64w); col 64 = softmax denominator)
      out_tok = av * recip(denom) broadcast        (1 recip + 1 tensor_tensor)
      outT = PE-transpose(out_tok)                 (4 transposes)
      fin [t128, 256] = outT.T @ Wo + bo           (5 MMs, N=256) -> DMA out
  Host unpacks [img, wx, u, t128, c] bf16 -> (32,56,56,256) f32.
"""

from contextlib import ExitStack

import ml_dtypes
import numpy as np

import concourse.bacc as bacc
import concourse.bass as bass
import concourse.tile as tile
from concourse import mybir
from concourse.masks import make_identity
from concourse.bass_utils import run_bass_kernel_spmd

P = 7
PP = 49
H = 8
D = 64
DIM = 256
INNER = 512
SCALE = D ** -0.5
IMGS_PER_CORE = 4
NCORES = 8
X = 56
NW = X // P      # 8 windows per axis
FP32 = mybir.dt.float32
BF16 = mybir.dt.bfloat16
NPBF16 = ml_dtypes.bfloat16
Exp = mybir.ActivationFunctionType.Exp


def build_bass(n_imgs=IMGS_PER_CORE):
    nc = bacc.Bacc("TRN2", target_bir_lowering=False, debug=False)

    fmp = nc.dram_tensor("fmp", [n_imgs, 2, 128, NW, NW, 64], BF16,
                         kind="ExternalInput").ap()
    wq = nc.dram_tensor("Wq", [2, 128, INNER], BF16, kind="ExternalInput").ap()
    wk = nc.dram_tensor("Wk", [2, 128, INNER], BF16, kind="ExternalInput").ap()
    wv = nc.dram_tensor("Wv", [2, 128, INNER], BF16, kind="ExternalInput").ap()
    wo = nc.dram_tensor("Wo", [4, 128, DIM], BF16, kind="ExternalInput").ap()
    bo = nc.dram_tensor("bo", [DIM], BF16, kind="ExternalInput").ap()
    out = nc.dram_tensor("out", [n_imgs, NW, 4, 128, DIM], BF16,
                         kind="ExternalOutput").ap()

    with tile.TileContext(nc) as tc:
        with ExitStack() as ctx:
            build_kernel(ctx, tc, out, fmp, wq, wk, wv, wo, bo, n_imgs)
    nc.compile()
    return nc


def build_kernel(ctx, tc, out, fmp, wq, wk, wv, wo, bo, n_imgs):
    nc = tc.nc
    consts = ctx.enter_context(tc.tile_pool(name="consts", bufs=1))
    sb = ctx.enter_context(tc.tile_pool(name="sb", bufs=3))
    ps = ctx.enter_context(tc.tile_pool(name="ps", bufs=4, space="PSUM"))

    ident = consts.tile([128, 128], BF16)
    make_identity(nc, ident[:])
    ones = consts.tile([1, 128], BF16)
    nc.gpsimd.memset(ones[:], 1.0)

    wq_s = consts.tile([128, 2, INNER], BF16)
    nc.sync.dma_start(out=wq_s[:], in_=wq.rearrange("kc ck n -> ck kc n"))
    wk_s = consts.tile([128, 2, INNER], BF16)
    nc.sync.dma_start(out=wk_s[:], in_=wk.rearrange("kc ck n -> ck kc n"))
    wv_s = consts.tile([128, 2, INNER], BF16)
    nc.sync.dma_start(out=wv_s[:], in_=wv.rearrange("kc ck n -> ck kc n"))
    wo_s = consts.tile([128, 4, DIM], BF16)
    nc.sync.dma_start(out=wo_s[:], in_=wo.rearrange("kc ck m -> ck kc m"))
    bo_s = consts.tile([1, DIM], BF16)
    nc.sync.dma_start(out=bo_s[:], in_=bo[None, :])

    def ps_slot(shape, dtype=FP32):
        # uniform 2-bank (4KB/partition) psum slots; view-slice to shape
        t = ps.tile([128, 4096 // mybir.dt.size(dtype)], dtype, tag="ps")
        n = int(np.prod(shape[1:]))
        v_ = t[:, 0:n]
        if len(shape) > 2:
            dims = " ".join(f"d{i}" for i in range(1, len(shape)))
            v_ = v_.rearrange(f"p ({dims}) -> p {dims}",
                              **{f"d{i}": shape[i] for i in range(1, len(shape) - 1)})
        return v_

    prev = None
    for img in range(n_imgs):
        for wx in range(NW):
            vhat, qT, kT = tile_qkv(nc, sb, ps_slot, fmp, wq_s, wk_s, wv_s,
                                    img, wx)
            for u in range(4):
                expS = attn_phase_s(nc, sb, ps_slot, qT, kT, u)
                if prev is not None:
                    out_phase(nc, sb, ps_slot, out, wo_s, bo_s, ident, ones,
                              prev)
                out_tok = attn_phase_av(nc, sb, ps_slot, expS, vhat[u])
                prev = (out_tok, img, wx, u)
    out_phase(nc, sb, ps_slot, out, wo_s, bo_s, ident, ones, prev)


def tile_qkv(nc, sb, ps_slot, fmp, wq_s, wk_s, wv_s, img, wx):
    # ---- load fT [c128, ck, t512] ----
    fT = sb.tile([128, 2, 512], BF16, tag="fT")
    nc.sync.dma_start(
        out=fT[:],
        in_=fmp[img, :, :, wx, :, :].rearrange("ck c wy t -> c ck (wy t)"))

    # ---- qT, kT: [n128, t512] x4 chunks ----
    qp = [ps_slot([128, 2, 512]) for _ in range(2)]   # slot holds nk, nk+1
    kp = [ps_slot([128, 2, 512]) for _ in range(2)]
    for half in range(2):
        for sub in range(2):
            nk = 2 * half + sub
            for kc in range(2):
                nc.tensor.matmul(qp[half][:, sub, :],
                                 wq_s[:, kc, 128 * nk:128 * nk + 128],
                                 fT[:, kc, :], start=(kc == 0), stop=(kc == 1))
        for sub in range(2):
            nk = 2 * half + sub
            for kc in range(2):
                nc.tensor.matmul(kp[half][:, sub, :],
                                 wk_s[:, kc, 128 * nk:128 * nk + 128],
                                 fT[:, kc, :], start=(kc == 0), stop=(kc == 1))
    qT = sb.tile([128, 4, 512], BF16, tag="qT")
    kT = sb.tile([128, 4, 512], BF16, tag="kT")
    # q/k evacuation on scalar; vector carries vhat/norm/outT
    nc.scalar.copy(qT[:, 0:2, :], qp[0][:])
    nc.scalar.copy(qT[:, 2:4, :], qp[1][:])
    nc.scalar.copy(kT[:, 0:2, :], kp[0][:])
    nc.scalar.copy(kT[:, 2:4, :], kp[1][:])

    # ---- v: per pair [t128, n512]; vhat adds ones col per head ----
    vhat = []
    for uh in range(2):
        vp = ps_slot([128, 2, 512])   # two pairs per slot? no: [t128, pairsub, n512]
        for us in range(2):
            u = 2 * uh + us
            for kc in range(2):
                nc.tensor.matmul(vp[:, us, :],
                                 fT[:, kc, 128 * u:128 * u + 128],
                                 wv_s[:, kc, :], start=(kc == 0), stop=(kc == 1))
        for us in range(2):
            u = 2 * uh + us
            vh = sb.tile([128, H, D + 1], BF16, tag=f"vhat{u}")
            nc.vector.tensor_copy(
                vh[:, :, 0:D], vp[:, us, :].rearrange("p (h d) -> p h d", h=H))
            nc.gpsimd.memset(vh[:, :, D:D + 1], 1.0)
            vhat.append(vh)

    return vhat, qT, kT


def attn_phase_s(nc, sb, ps_slot, qT, kT, u):
    """Quad-tiled S^T + one full-width exp. Returns (sp is consumed), expS."""
    # sp[j128, hp, nkx8, i64]: hp stride = 8*64*4B = 2KB -> bank-separated
    sp = ps_slot([128, 2, 8, 64])
    for nk in range(4):
        for hp in range(2):
            for w in range(2):
                o = 128 * u + 64 * w
                nc.tensor.matmul(
                    sp[64 * w:64 * w + 64, hp, nk, :],
                    kT[64 * hp:64 * hp + 64, nk, o:o + 64],
                    qT[64 * hp:64 * hp + 64, nk, o:o + 64],
                    start=True, stop=True, tile_position=(64 * hp, 64 * w))
    expS = sb.tile([128, 2, 4, 64], BF16, tag="expS")
    nc.scalar.activation(expS[:], sp[:, :, 0:4, :], Exp, scale=SCALE)
    return expS


def attn_phase_av(nc, sb, ps_slot, expS, vh):
    """AV + denominator, normalize. Returns out_tok [i128, h, d] bf16."""
    av = ps_slot([128, H, 128])
    for h in range(H):
        nk, hp = h // 2, h % 2
        for w in range(2):
            nc.tensor.matmul(
                av[64 * w:64 * w + 64, h, 0:D + 1],
                expS[64 * w:64 * w + PP, hp, nk, :],
                vh[64 * w:64 * w + PP, h, :],
                start=True, stop=True, tile_position=(64 * w, 64 * w))
    recd = sb.tile([128, H], FP32, tag="recd")
    nc.vector.reciprocal(recd[:], av[:, :, D])
    out_tok = sb.tile([128, H, D], BF16, tag="out_tok")
    nc.vector.tensor_tensor(
        out=out_tok[:], in0=av[:, :, 0:D],
        in1=recd[:].unsqueeze(2).broadcast_to([128, H, D]),
        op=mybir.AluOpType.mult)
    return out_tok


def out_phase(nc, sb, ps_slot, out, wo_s, bo_s, ident, ones, prev):
    """Transpose + out-projection + store for a previous pair's out_tok."""
    out_tok, img, wx, u = prev
    tpfin = ps_slot([128, 1024])
    tp = tpfin[:, 0:256].bitcast(BF16).rearrange("p (nk t) -> p nk t", nk=4)
    fin = tpfin[:, 256:512]
    ot2 = out_tok[:].rearrange("p h d -> p (h d)")
    for nk in range(4):
        nc.tensor.transpose(tp[:, nk, :], ot2[:, 128 * nk:128 * nk + 128],
                            ident[:])
    outT = sb.tile([128, 4, 128], BF16, tag="outT")
    nc.vector.tensor_copy(outT[:], tp[:])
    for nk in range(4):
        nc.tensor.matmul(fin[:], outT[:, nk, :], wo_s[:, nk, :],
                         start=(nk == 0), stop=False)
    nc.tensor.matmul(fin[:], ones[0:1, 0:128], bo_s[:], start=False, stop=True)
    fo = sb.tile([128, DIM], BF16, tag="fo")
    nc.scalar.copy(fo[:], fin[:])
    nc.sync.dma_start(out=out[img, wx, u], in_=fo[:])


_CACHED = {}


def _get_nc():
    if "nc" not in _CACHED:
        _CACHED["nc"] = build_bass()
    return _CACHED["nc"]


def _marshal_fmap(fmap):
    b = fmap.shape[0]
    A = fmap.astype(NPBF16)
    A = A.reshape(b, NW, P, NW, P, DIM).transpose(0, 5, 1, 3, 2, 4)
    A = np.ascontiguousarray(A).reshape(b, DIM, NW, NW, PP)
    T = np.zeros((b, DIM, NW, NW, 64), dtype=NPBF16)
    T[..., :PP] = A
    return T.reshape(b, 2, 128, NW, NW, 64)


def _unmarshal_out(O, b):
    # O: [b, wx, u, t2(=w*64+t), c] bf16
    O = O.reshape(b, NW, 4, 2, 64, DIM)[:, :, :, :, :PP, :]
    O = O.reshape(b, NW, NW, P, P, DIM)         # img, wx, wy(=2u+w), p1, p2, c
    O = O.transpose(0, 1, 3, 2, 4, 5).reshape(b, X, X, DIM)
    return O.astype(np.float32)


def kernel(fmap, Wq, Wkv, Wo, bo, _trace=False, _trace_kwargs=None):
    fmp = _marshal_fmap(np.ascontiguousarray(fmap))
    Wq_ = np.ascontiguousarray(Wq).astype(NPBF16).reshape(2, 128, INNER)
    Wk_ = np.ascontiguousarray(Wkv[:, :INNER]).astype(NPBF16).reshape(2, 128, INNER)
    Wv_ = np.ascontiguousarray(Wkv[:, INNER:]).astype(NPBF16).reshape(2, 128, INNER)
    Wo_ = np.ascontiguousarray(Wo).astype(NPBF16).reshape(4, 128, DIM)
    bo_ = np.ascontiguousarray(bo).astype(NPBF16)
    nc = _get_nc()
    in_maps = []
    for c in range(NCORES):
        in_maps.append({
            "fmp": fmp[IMGS_PER_CORE * c:IMGS_PER_CORE * (c + 1)],
            "Wq": Wq_, "Wk": Wk_, "Wv": Wv_, "Wo": Wo_, "bo": bo_,
        })
    res = run_bass_kernel_spmd(nc, in_maps, core_ids=list(range(NCORES)),
                               trace=_trace, **(_trace_kwargs or {}))
    outs = [_unmarshal_out(r["out"], IMGS_PER_CORE) for r in res.results]
    full = np.concatenate(outs, axis=0)
    if _trace:
        return full, res
    return full
